# revision 51
# baseline (speedup 1.0000x reference)
"""Local sliding-window attention block (MQA + partial RoPE) on 8 TRN2 cores.

Sharding: 2 batches x 4 sequence chunks of 512 queries each (no collectives).
Each core computes q/k/v projections for its chunk (keys include a 512-token
halo), windowed attention (window=512, causal), and the o-projection for its
own query rows; host unshard is pure concatenation.

Projections (Q/K/V/O) run as error-compensated fp8-e4m3 DoubleRow matmuls:
operands are split hi/lo (x = x_hi + x_lo, W*64 = W_hi + W_lo) and the three
product terms hi*hi, hi*lo, lo*hi are paired two-per-instruction across
adjacent contraction slices, giving 0.75x the bf16 cycle count at ~bf16
accuracy. All splits are precomputed host-side except the attention output's
(one Act copy + one DVE sub per head). Scores/PV stay bf16.

Softmax denominators come from all-ones-lhsT matmuls accumulating into a
spare slice of the score PSUM tile (replicated across partitions), replacing
the DVE reduce + GPSIMD all-reduce. Sliding-window masks apply to only the
two boundary key tiles; first-chunk (no-halo) cores are handled with pure
data: all-ones m0, a constant correction lhsT for the sig-0 denominator
matmul, and zero padding in x.

DMAs are spread across the SP, Activation and Pool queues with >=512B
descriptors; dummy matmuls warm the PE clock during the initial load.
"""

import numpy as np
import ml_dtypes

BF16 = ml_dtypes.bfloat16
F8 = ml_dtypes.float8_e4m3

B, L, D = 2, 2048, 2048
H, HD = 16, 128
ROPE_DIMS, HALF = 64, 32
WINDOW = 512
ROPE_BASE = 10000.0
WSCALE = 64.0                    # weight pre-scale for fp8 range
EXP_SCALE = HD ** -0.5 / (WSCALE * WSCALE)

CHUNK = 512            # queries per core
NK = 1024              # keys (incl. halo) per core
NQT = CHUNK // 128     # 4 local query tiles
NSIG = 5               # key tiles in window per query tile
NDT = D // 128         # 16 contraction tiles over embedding dim
NDP = NDT // 2         # 8 contraction tile-pairs
N_WARMUP = 230  # dummy matmuls to hold PE busy through the p-state ramp

_PROGRAM = None


def _rope(nc, mybir, pool_tmp, out_bf, ps, cos2, sin2m, fp32, bf16):
    """out[0:64] = rotary(ps[0:64]); out[64:128] = ps[64:128]. ps fp32, out bf16.

    cos2 is [64, n] with rows [0:32]==[32:64]==cos(theta); sin2m has rows
    [0:32]==-sin(theta), [32:64]==+sin(theta). Engines can't read across
    partitions, so the half-swap (x2 into rows 0:32, x1 into rows 32:64)
    goes through two partition-shifting DMAs (issued on the Pool queue);
    then out[0:64] = ps[0:64]*cos2 + swapped*sin2m is partition-aligned.
    """
    n = cos2.shape[-1]
    sb64 = pool_tmp.tile([ROPE_DIMS, n], bf16, tag="rope_sb64")
    nc.scalar.copy(sb64, ps[0:ROPE_DIMS])
    ss = pool_tmp.tile([ROPE_DIMS, n], bf16, tag="rope_ss")
    nc.gpsimd.dma_start(out=ss[0:HALF], in_=sb64[HALF:ROPE_DIMS])
    nc.gpsimd.dma_start(out=ss[HALF:ROPE_DIMS], in_=sb64[0:HALF])
    mcos = pool_tmp.tile([ROPE_DIMS, n], fp32, tag="rope_mcos")
    nc.vector.tensor_mul(mcos, ps[0:ROPE_DIMS], cos2)
    mss = pool_tmp.tile([ROPE_DIMS, n], fp32, tag="rope_mss")
    nc.vector.tensor_mul(mss, ss, sin2m)
    nc.vector.tensor_add(out_bf[0:ROPE_DIMS], mcos, mss)
    nc.vector.tensor_copy(out_bf[ROPE_DIMS:HD], ps[ROPE_DIMS:HD])


def _comp_proj(nc, ps, w8, xhi, xlo, cols, n_start, n_stop, DR):
    """Accumulate the compensated-fp8 projection into psum `ps`.

    w8: weight tile [128, NDP, 2(hi/lo), 2(dt pair), M] fp8.
    xhi/xlo: lists of NDP tiles [128, 2(dt pair), NK] fp8.
    For each dt-pair j, three DoubleRow matmuls:
      hi(W) x hi(x),  lo(W) x hi(x),  hi(W) x lo(x).
    """
    for j in range(NDP):
        xh = xhi[j][:, :, cols]
        first = n_start and j == 0
        nc.tensor.matmul(ps, w8[:, j, 0, :, :], xh,
                         start=first, stop=False, perf_mode=DR)
        nc.tensor.matmul(ps, w8[:, j, 1, :, :], xh,
                         start=False, stop=False, perf_mode=DR)
    for j in range(NDP):
        last = n_stop and j == NDP - 1
        nc.tensor.matmul(ps, w8[:, j, 0, :, :], xlo[j][:, :, cols],
                         start=False, stop=last, perf_mode=DR)


def _build_program():
    from contextlib import ExitStack
    import concourse.bass as bass
    import concourse.mybir as mybir
    import concourse.tile as tile
    from concourse import bacc

    fp32 = mybir.dt.float32
    bf16 = mybir.dt.bfloat16
    f8 = mybir.dt.float8e4
    AF = mybir.ActivationFunctionType
    ALU = mybir.AluOpType
    DR = mybir.MatmulPerfMode.DoubleRow

    nc = bacc.Bacc(None, target_bir_lowering=False)

    xhi_d = nc.dram_tensor("xhi", [NDP, 128, 2, NK], f8, kind="ExternalInput")
    xlo_d = nc.dram_tensor("xlo", [NDP, 128, 2, NK], f8, kind="ExternalInput")
    wq_d = nc.dram_tensor("wq8", [H, 128, NDP, 2, 2, HD], f8, kind="ExternalInput")
    wk_d = nc.dram_tensor("wk8", [128, NDP, 2, 2, HD], f8, kind="ExternalInput")
    wv_d = nc.dram_tensor("wv8", [128, NDP, 2, 2, HD], f8, kind="ExternalInput")
    wohi_d = nc.dram_tensor("wohi", [4, 128, H, 512], f8, kind="ExternalInput")
    wolo_d = nc.dram_tensor("wolo", [4, 128, NDP, 2, 512], f8, kind="ExternalInput")
    bo_d = nc.dram_tensor("bo", [1, D], fp32, kind="ExternalInput")
    cos_d = nc.dram_tensor("cosT", [ROPE_DIMS, NK], fp32, kind="ExternalInput")
    sin_d = nc.dram_tensor("sinT", [ROPE_DIMS, NK], fp32, kind="ExternalInput")
    msk_d = nc.dram_tensor("masks", [128, 2, 128], bf16, kind="ExternalInput")
    corr_d = nc.dram_tensor("corrA", [128, NQT, 128], bf16, kind="ExternalInput")
    ones_d = nc.dram_tensor("ones", [128, 128], bf16, kind="ExternalInput")
    out_d = nc.dram_tensor("out", [CHUNK, D], fp32, kind="ExternalOutput")

    with tile.TileContext(nc) as tc, ExitStack() as ctx:
        p_const = ctx.enter_context(tc.tile_pool(name="const", bufs=1))
        p_x = ctx.enter_context(tc.tile_pool(name="x", bufs=1))
        p_kv = ctx.enter_context(tc.tile_pool(name="kv", bufs=1))
        p_wq = ctx.enter_context(tc.tile_pool(name="wq", bufs=3))
        p_qt = ctx.enter_context(tc.tile_pool(name="qt", bufs=4))
        p_es = ctx.enter_context(tc.tile_pool(name="es", bufs=6))
        p_red = ctx.enter_context(tc.tile_pool(name="red", bufs=2))
        p_tmp = ctx.enter_context(tc.tile_pool(name="tmp", bufs=4))
        p_otn = ctx.enter_context(tc.tile_pool(name="otn", bufs=1))
        p_wo = ctx.enter_context(tc.tile_pool(name="wo", bufs=4))
        p_ob = ctx.enter_context(tc.tile_pool(name="ob", bufs=3))

        # ---- persistent loads, spread across the three DMA queues ----
        # Activation queue: needed-first order (wk -> x tail -> wv -> cos/sin)
        wk_sb = p_const.tile([128, NDP, 2, 2, HD], f8, tag="wk")
        nc.scalar.dma_start(out=wk_sb, in_=wk_d[:])

        # x hi tiles: one tile per dt-pair, one DMA per dt slice (SP + Act).
        xhi = []
        xlo = []
        for j in range(NDP):
            xhi.append(p_x.tile([128, 2, NK], f8, tag=f"xhi{j}", name=f"xhi{j}"))
            xlo.append(p_x.tile([128, 2, NK], f8, tag=f"xlo{j}", name=f"xlo{j}"))
        xq = {0: nc.sync, 1: nc.sync, 2: nc.scalar, 3: nc.gpsimd,
              4: nc.sync, 5: nc.sync, 6: nc.scalar, 7: nc.gpsimd}
        for j in range(NDP):
            xq[j].dma_start(out=xhi[j][:, 0, :], in_=xhi_d[j, :, 0, :])
            xq[j].dma_start(out=xhi[j][:, 1, :], in_=xhi_d[j, :, 1, :])

        wv_sb = p_const.tile([128, NDP, 2, 2, HD], f8, tag="wv")
        nc.scalar.dma_start(out=wv_sb, in_=wv_d[:])
        cos_sb = p_const.tile([ROPE_DIMS, NK], fp32, tag="cos")
        nc.scalar.dma_start(out=cos_sb, in_=cos_d[:])
        sin_sb = p_const.tile([ROPE_DIMS, NK], fp32, tag="sin")
        nc.scalar.dma_start(out=sin_sb, in_=sin_d[:])

        for j in range(NDP):
            nc.gpsimd.dma_start(out=xlo[j], in_=xlo_d[j])

        # Pool queue: small constants.
        msk_sb = p_const.tile([128, 2, 128], bf16, tag="msk")
        nc.gpsimd.dma_start(out=msk_sb, in_=msk_d[:])
        corr_sb = p_const.tile([128, NQT, 128], bf16, tag="corr")
        nc.gpsimd.dma_start(out=corr_sb, in_=corr_d[:])
        ones_sb = p_const.tile([128, 128], bf16, tag="ones")
        nc.gpsimd.dma_start(out=ones_sb, in_=ones_d[:])
        bias_sb = p_const.tile([128, D], fp32, tag="bias")

        # ---- PE warm-up: tiny dummy matmuls from t~0 to beat the clock ramp
        zt = p_const.tile([128, 16], bf16, tag="zt")
        nc.vector.memset(zt, 0.0)

        # ---- K^T (RoPE'd) and V projections ----
        kt = p_kv.tile([128, NK], bf16, tag="kt")
        v_sb = []
        for s in range(NK // 128):
            v_sb.append(p_kv.tile([128, HD], bf16, tag=f"v{s}", name=f"v{s}"))

        with (
            tc.tile_pool(name="ps_kv", bufs=2, space=bass.MemorySpace.PSUM) as ps_kv,
            tc.tile_pool(name="ps_vv", bufs=5, space=bass.MemorySpace.PSUM) as ps_vv,
            tc.tile_pool(name="ps_dd", bufs=1, space=bass.MemorySpace.PSUM) as ps_dd,
        ):
            psd = ps_dd.tile([16, 16], fp32, tag="ps_dummy")
            for i in range(N_WARMUP):
                nc.tensor.matmul(psd, zt, zt, start=True, stop=True)

            # K: both blocks' hi-matmuls first (xhi-paced), then the xlo
            # corrections — pushes the xlo DMA demand later into the load.
            kps = []
            for nh in range(NK // 512):
                ps = ps_kv.tile([128, 512], fp32, tag="ps_kv")
                cols = slice(nh * 512, (nh + 1) * 512)
                for j in range(NDP):
                    xh = xhi[j][:, :, cols]
                    nc.tensor.matmul(ps, wk_sb[:, j, 0, :, :], xh,
                                     start=(j == 0), stop=False, perf_mode=DR)
                    nc.tensor.matmul(ps, wk_sb[:, j, 1, :, :], xh,
                                     start=False, stop=False, perf_mode=DR)
                kps.append(ps)
            for nh in range(NK // 512):
                cols = slice(nh * 512, (nh + 1) * 512)
                for j in range(NDP):
                    nc.tensor.matmul(kps[nh], wk_sb[:, j, 0, :, :],
                                     xlo[j][:, :, cols],
                                     start=False, stop=(j == NDP - 1),
                                     perf_mode=DR)
                _rope(nc, mybir, p_tmp, kt[:, cols], kps[nh],
                      cos_sb[:, cols], sin_sb[:, cols], fp32, bf16)

            for s in range(NK // 128):
                psv = ps_vv.tile([128, HD], fp32, tag="ps_v")
                cols = slice(s * 128, (s + 1) * 128)
                # V = x^T W: stationary = x slices, moving = Wv slices.
                for j in range(NDP):
                    xh = xhi[j][:, :, cols]
                    nc.tensor.matmul(psv, xh, wv_sb[:, j, 0, :, :],
                                     start=(j == 0), stop=False, perf_mode=DR)
                    nc.tensor.matmul(psv, xh, wv_sb[:, j, 1, :, :],
                                     start=False, stop=False, perf_mode=DR)
                for j in range(NDP):
                    nc.tensor.matmul(psv, xlo[j][:, :, cols], wv_sb[:, j, 0, :, :],
                                     start=False, stop=(j == NDP - 1), perf_mode=DR)
                nc.scalar.activation(v_sb[s], psv, AF.Copy, scale=1.0 / WSCALE)

        # ---- per-head attention ----
        otn8 = p_otn.tile([128, H, 2, CHUNK], f8, tag="otn8")
        qcols = slice(CHUNK, NK)

        with (
            tc.tile_pool(name="ps_q", bufs=2, space=bass.MemorySpace.PSUM) as ps_qp,
            tc.tile_pool(name="ps_s", bufs=2, space=bass.MemorySpace.PSUM) as ps_sp,
            tc.tile_pool(name="ps_o", bufs=2, space=bass.MemorySpace.PSUM) as ps_op,
        ):
            def _normalize(h, otp, rview):
                # deferred one head: off the inter-head critical path
                rflat = rview.rearrange("p t q -> p (t q)")
                tmp32 = p_tmp.tile([128, CHUNK], fp32, tag="otn_tmp", name="otn_tmp")
                nc.vector.tensor_mul(tmp32, otp, rflat)
                nc.scalar.copy(otn8[:, h, 0, :], tmp32)
                nc.gpsimd.tensor_sub(otn8[:, h, 1, :], tmp32, otn8[:, h, 0, :])

            def _qproj(h):
                wq_h = p_wq.tile([128, NDP, 2, 2, HD], f8, tag="wq", name="wq_h")
                nc.sync.dma_start(out=wq_h, in_=wq_d[h])
                psq = ps_qp.tile([128, CHUNK], fp32, tag="ps_q", name="psq")
                _comp_proj(nc, psq, wq_h, xhi, xlo, qcols, True, True, DR)
                return psq

            pending = None
            psq = _qproj(0)
            for h in range(H):
                qt = p_qt.tile([128, CHUNK], bf16, tag="qt")
                _rope(nc, mybir, p_tmp, qt, psq,
                      cos_sb[:, qcols], sin_sb[:, qcols], fp32, bf16)
                if h + 1 < H:
                    psq = _qproj(h + 1)

                otp = ps_op.tile([128, CHUNK], fp32, tag="ps_o")
                rview = p_red.tile([128, NQT, 128], fp32, tag="recip")
                for t in range(NQT):
                    pss = ps_sp.tile([128, NSIG + 1, 128], fp32, tag="ps_s")
                    qsl = qt[:, t * 128:(t + 1) * 128]
                    for sig in range(NSIG):
                        s = t + sig
                        nc.tensor.matmul(
                            pss[:, sig, :], kt[:, s * 128:(s + 1) * 128], qsl,
                            start=True, stop=True,
                        )
                    es = p_es.tile([128, NSIG, 128], bf16, tag="es")
                    nc.scalar.activation(es, pss[:, 0:NSIG, :], AF.Exp,
                                         scale=EXP_SCALE)
                    # boundary masks on sig 0 and sig 4 only (one strided op)
                    esm = es[:]
                    es04 = bass.AP(tensor=esm.tensor, offset=esm.offset,
                                   ap=[list(esm.ap[0]), [4 * 128, 2], [1, 128]])
                    nc.gpsimd.tensor_mul(es04, es04, msk_sb)
                    # denominators: all-ones lhsT (corrA for sig 0) accumulated
                    # into the spare psum slice, replicated across partitions.
                    # middle sigs (1..3) don't depend on the mask op, so they
                    # issue first, hiding the mask latency.
                    for i, sig in enumerate((1, 2, 3, 0, 4)):
                        nc.tensor.matmul(pss[:, NSIG, :],
                                         corr_sb[:, t, :] if sig == 0 else ones_sb,
                                         es[:, sig, :],
                                         start=(i == 0), stop=(i == NSIG - 1))
                    nc.vector.reciprocal(rview[:, t, :], pss[:, NSIG, :])
                    for i, sig in enumerate((1, 2, 3, 0, 4)):
                        nc.tensor.matmul(
                            otp[:, t * 128:(t + 1) * 128],
                            v_sb[t + sig], es[:, sig, :],
                            start=(i == 0), stop=(i == NSIG - 1),
                        )
                if pending is not None:
                    _normalize(*pending)
                pending = (h, otp, rview)
            _normalize(*pending)

            # ---- o-projection + bias (same pool scope: overlaps last heads)
            # chunk big weight DMAs so they never hold the SP queue > ~1us
            for nb in range(4):
                nc.sync.dma_start(
                    out=bias_sb[:, nb * 512:(nb + 1) * 512],
                    in_=bass.AP(tensor=bo_d, offset=nb * 512,
                                ap=[[0, 128], [1, 512]]),
                )
            for n in range(D // 512):
                wo_hi = p_wo.tile([128, H, 512], f8, tag="wo_hi")
                wo_lo = p_wo.tile([128, NDP, 2, 512], f8, tag="wo_lo")
                for q4 in range(4):
                    nc.sync.dma_start(out=wo_hi[:, 4 * q4:4 * q4 + 4, :],
                                      in_=wohi_d[n, :, 4 * q4:4 * q4 + 4, :])
                    nc.sync.dma_start(out=wo_lo[:, 2 * q4:2 * q4 + 2, :, :],
                                      in_=wolo_d[n, :, 2 * q4:2 * q4 + 2, :, :])
                ncols = slice(n * 512, (n + 1) * 512)
                for t in range(NQT):
                    pso = ps_op.tile([128, CHUNK], fp32, tag="ps_o", name="pso")
                    tc_ = slice(t * 128, (t + 1) * 128)
                    for j in range(NDP):
                        whp = wo_hi[:, 2 * j:2 * j + 2, :]
                        nc.tensor.matmul(pso, otn8[:, 2 * j:2 * j + 2, 0, tc_],
                                         whp, start=(j == 0), stop=False,
                                         perf_mode=DR)
                        nc.tensor.matmul(pso, otn8[:, 2 * j:2 * j + 2, 1, tc_],
                                         whp, start=False, stop=False,
                                         perf_mode=DR)
                    for j in range(NDP):
                        nc.tensor.matmul(pso, otn8[:, 2 * j:2 * j + 2, 0, tc_],
                                         wo_lo[:, j, :, :], start=False,
                                         stop=(j == NDP - 1), perf_mode=DR)
                    ob = p_ob.tile([128, 512], fp32, tag="ob")
                    rows = slice(t * 128, (t + 1) * 128)
                    if (n, t) != (3, 3):
                        nc.vector.scalar_tensor_tensor(
                            ob, pso, 1.0 / WSCALE, bias_sb[:, ncols],
                            op0=ALU.mult, op1=ALU.add,
                        )
                        if (n + t) % 2 == 0:
                            nc.sync.dma_start(out=out_d[rows, ncols], in_=ob)
                        else:
                            nc.gpsimd.dma_start(out=out_d[rows, ncols], in_=ob)
                    else:
                        # final tile: split the drain so the last DMA starts
                        # as early as possible
                        for half, q in ((0, nc.sync), (1, nc.gpsimd)):
                            hc = slice(half * 256, (half + 1) * 256)
                            oc = slice(n * 512 + half * 256,
                                       n * 512 + (half + 1) * 256)
                            nc.vector.scalar_tensor_tensor(
                                ob[:, hc], pso[:, hc], 1.0 / WSCALE,
                                bias_sb[:, oc], op0=ALU.mult, op1=ALU.add,
                            )
                            q.dma_start(out=out_d[rows, oc], in_=ob[:, hc])

    nc.compile()
    return nc


def _get_program():
    global _PROGRAM
    if _PROGRAM is None:
        _PROGRAM = _build_program()
    return _PROGRAM


def _q8(a):
    return np.clip(a, -240.0, 240.0).astype(F8)


def _split8(a):
    hi = _q8(a)
    lo = _q8(a - hi.astype(np.float32))
    return hi, lo


def _make_in_maps(x, Wq, Wk, Wv, Wo, bo):
    x = np.asarray(x, np.float32)
    bo_f = np.ascontiguousarray(np.asarray(bo, np.float32).reshape(1, D))

    # --- weights (shared across cores) ---
    qhi, qlo = _split8(WSCALE * np.asarray(Wq, np.float32))
    khi, klo = _split8(WSCALE * np.asarray(Wk, np.float32))
    vhi, vlo = _split8(WSCALE * np.asarray(Wv, np.float32))
    ohi, olo = _split8(WSCALE * np.asarray(Wo, np.float32))

    def warr(hi, lo, M):
        # [D, M] pair -> [128, NDP, 2(hi/lo), 2(dt pair), M]
        w = np.empty((128, NDP, 2, 2, M), F8)
        hi4 = hi.reshape(NDT, 128, M)
        lo4 = lo.reshape(NDT, 128, M)
        for j in range(NDP):
            w[:, j, 0, 0] = hi4[2 * j]
            w[:, j, 0, 1] = hi4[2 * j + 1]
            w[:, j, 1, 0] = lo4[2 * j]
            w[:, j, 1, 1] = lo4[2 * j + 1]
        return np.ascontiguousarray(w)

    wq8 = np.stack([warr(qhi[:, h * HD:(h + 1) * HD], qlo[:, h * HD:(h + 1) * HD], HD)
                    for h in range(H)])
    wk8 = warr(khi, klo, HD)
    wv8 = warr(vhi, vlo, HD)

    wohi = np.empty((4, 128, H, 512), F8)
    wolo = np.empty((4, 128, NDP, 2, 512), F8)
    ohi4 = ohi.reshape(H, 128, D)
    olo4 = olo.reshape(H, 128, D)
    for n in range(4):
        cs = slice(n * 512, (n + 1) * 512)
        for h in range(H):
            wohi[n, :, h] = ohi4[h][:, cs]
        for j in range(NDP):
            wolo[n, :, j, 0] = olo4[2 * j][:, cs]
            wolo[n, :, j, 1] = olo4[2 * j + 1][:, cs]
    wohi = np.ascontiguousarray(wohi)
    wolo = np.ascontiguousarray(wolo)

    inv_freq = np.exp(
        -np.log(np.float32(ROPE_BASE))
        * (np.arange(0, ROPE_DIMS, 2, dtype=np.float32) / np.float32(ROPE_DIMS))
    ).astype(np.float32)

    ones = np.ones((128, 128), BF16)
    # masks: es tile is [key r (partitions), q]; m0 strict upper (r>q),
    # m4 causal lower (r<=q)
    r = np.arange(128)[:, None]
    qi = np.arange(128)[None, :]
    m0_tri = (r > qi).astype(BF16)
    m4 = (r <= qi).astype(BF16)

    in_maps = []
    for c in range(8):
        b, g = divmod(c, 4)
        k_start = 512 * g - 512
        xs = np.zeros((NK, D), np.float32)
        lo_ = max(0, k_start)
        xs[lo_ - k_start:] = x[b, lo_:k_start + NK]
        xT = np.ascontiguousarray(xs.T)                    # [D, NK]
        xh, xl = _split8(xT)
        xhi_a = np.ascontiguousarray(xh.reshape(NDP, 2, 128, NK).transpose(0, 2, 1, 3))
        xlo_a = np.ascontiguousarray(xl.reshape(NDP, 2, 128, NK).transpose(0, 2, 1, 3))

        pos = (k_start + np.arange(NK)).astype(np.float32)
        theta = pos[None, :] * inv_freq[:, None]           # [32, NK]
        cos2 = np.ascontiguousarray(
            np.concatenate([np.cos(theta)] * 2, axis=0).astype(np.float32))
        sin2 = np.ascontiguousarray(
            np.concatenate([-np.sin(theta), np.sin(theta)], axis=0).astype(np.float32))

        msk = np.empty((128, 2, 128), BF16)
        msk[:, 0] = np.ones((128, 128), BF16) if g == 0 else m0_tri
        msk[:, 1] = m4
        corr = np.empty((128, NQT, 128), BF16)
        for t in range(NQT):
            corr[:, t] = np.float32(t - 3) if g == 0 else 1.0

        in_maps.append({
            "xhi": xhi_a, "xlo": xlo_a, "wq8": wq8, "wk8": wk8, "wv8": wv8,
            "wohi": wohi, "wolo": wolo, "bo": bo_f, "cosT": cos2, "sinT": sin2,
            "masks": np.ascontiguousarray(msk), "corrA": np.ascontiguousarray(corr),
            "ones": ones,
        })
    return in_maps


def _unshard(results):
    out = np.zeros((B, L, D), np.float32)
    for c in range(8):
        b, g = divmod(c, 4)
        out[b, CHUNK * g:CHUNK * (g + 1)] = results[c]["out"]
    return out


def kernel(x, Wq, Wk, Wv, Wo, bo):
    from concourse.bass_utils import run_bass_kernel_spmd

    nc = _get_program()
    in_maps = _make_in_maps(x, Wq, Wk, Wv, Wo, bo)
    res = run_bass_kernel_spmd(nc, in_maps, core_ids=list(range(8)))
    return _unshard(res.results)



# revision 52
# speedup vs baseline: 1.0126x; 1.0126x over previous
"""Local sliding-window attention block (MQA + partial RoPE) on 8 TRN2 cores.

Sharding: 2 batches x 4 sequence chunks of 512 queries each (no collectives).
Each core computes q/k/v projections for its chunk (keys include a 512-token
halo), windowed attention (window=512, causal), and the o-projection for its
own query rows; host unshard is pure concatenation.

Projections (Q/K/V/O) run as error-compensated fp8-e4m3 DoubleRow matmuls:
operands are split hi/lo (x = x_hi + x_lo, W*64 = W_hi + W_lo) and the three
product terms hi*hi, hi*lo, lo*hi are paired two-per-instruction across
adjacent contraction slices, giving 0.75x the bf16 cycle count at ~bf16
accuracy. All splits are precomputed host-side except the attention output's
(one Act copy + one DVE sub per head). Scores/PV stay bf16.

Softmax denominators come from all-ones-lhsT matmuls accumulating into a
spare slice of the score PSUM tile (replicated across partitions), replacing
the DVE reduce + GPSIMD all-reduce. Sliding-window masks apply to only the
two boundary key tiles; first-chunk (no-halo) cores are handled with pure
data: all-ones m0, a constant correction lhsT for the sig-0 denominator
matmul, and zero padding in x.

DMAs are spread across the SP, Activation and Pool queues with >=512B
descriptors; dummy matmuls warm the PE clock during the initial load.

CoreSim cost model: ~156 us per core (PE 94% busy) vs 225 us for the
bf16 baseline; measured rel err vs the fp32 reference: 7.0e-3.
"""

import numpy as np
import ml_dtypes

BF16 = ml_dtypes.bfloat16
F8 = ml_dtypes.float8_e4m3

B, L, D = 2, 2048, 2048
H, HD = 16, 128
ROPE_DIMS, HALF = 64, 32
WINDOW = 512
ROPE_BASE = 10000.0
WSCALE = 64.0                    # weight pre-scale for fp8 range
EXP_SCALE = HD ** -0.5 / (WSCALE * WSCALE)

CHUNK = 512            # queries per core
NK = 1024              # keys (incl. halo) per core
NQT = CHUNK // 128     # 4 local query tiles
NSIG = 5               # key tiles in window per query tile
NDT = D // 128         # 16 contraction tiles over embedding dim
NDP = NDT // 2         # 8 contraction tile-pairs
N_WARMUP = 230  # dummy matmuls to hold PE busy through the p-state ramp

_PROGRAM = None


def _rope(nc, mybir, pool_tmp, out_bf, ps, cos2, sin2m, fp32, bf16):
    """out[0:64] = rotary(ps[0:64]); out[64:128] = ps[64:128]. ps fp32, out bf16.

    cos2 is [64, n] with rows [0:32]==[32:64]==cos(theta); sin2m has rows
    [0:32]==-sin(theta), [32:64]==+sin(theta). Engines can't read across
    partitions, so the half-swap (x2 into rows 0:32, x1 into rows 32:64)
    goes through two partition-shifting DMAs (issued on the Pool queue);
    then out[0:64] = ps[0:64]*cos2 + swapped*sin2m is partition-aligned.
    """
    n = cos2.shape[-1]
    sb64 = pool_tmp.tile([ROPE_DIMS, n], bf16, tag="rope_sb64")
    nc.scalar.copy(sb64, ps[0:ROPE_DIMS])
    ss = pool_tmp.tile([ROPE_DIMS, n], bf16, tag="rope_ss")
    nc.gpsimd.dma_start(out=ss[0:HALF], in_=sb64[HALF:ROPE_DIMS])
    nc.gpsimd.dma_start(out=ss[HALF:ROPE_DIMS], in_=sb64[0:HALF])
    mcos = pool_tmp.tile([ROPE_DIMS, n], fp32, tag="rope_mcos")
    nc.vector.tensor_mul(mcos, ps[0:ROPE_DIMS], cos2)
    mss = pool_tmp.tile([ROPE_DIMS, n], fp32, tag="rope_mss")
    nc.vector.tensor_mul(mss, ss, sin2m)
    nc.vector.tensor_add(out_bf[0:ROPE_DIMS], mcos, mss)
    nc.vector.tensor_copy(out_bf[ROPE_DIMS:HD], ps[ROPE_DIMS:HD])


def _comp_proj(nc, ps, w8, xhi, xlo, cols, n_start, n_stop, DR):
    """Accumulate the compensated-fp8 projection into psum `ps`.

    w8: weight tile [128, NDP, 2(hi/lo), 2(dt pair), M] fp8.
    xhi/xlo: lists of NDP tiles [128, 2(dt pair), NK] fp8.
    For each dt-pair j, three DoubleRow matmuls:
      hi(W) x hi(x),  lo(W) x hi(x),  hi(W) x lo(x).
    """
    for j in range(NDP):
        xh = xhi[j][:, :, cols]
        first = n_start and j == 0
        nc.tensor.matmul(ps, w8[:, j, 0, :, :], xh,
                         start=first, stop=False, perf_mode=DR)
        nc.tensor.matmul(ps, w8[:, j, 1, :, :], xh,
                         start=False, stop=False, perf_mode=DR)
    for j in range(NDP):
        last = n_stop and j == NDP - 1
        nc.tensor.matmul(ps, w8[:, j, 0, :, :], xlo[j][:, :, cols],
                         start=False, stop=last, perf_mode=DR)


def _build_program():
    from contextlib import ExitStack
    import concourse.bass as bass
    import concourse.mybir as mybir
    import concourse.tile as tile
    from concourse import bacc

    fp32 = mybir.dt.float32
    bf16 = mybir.dt.bfloat16
    f8 = mybir.dt.float8e4
    AF = mybir.ActivationFunctionType
    ALU = mybir.AluOpType
    DR = mybir.MatmulPerfMode.DoubleRow

    nc = bacc.Bacc(None, target_bir_lowering=False)

    xhi_d = nc.dram_tensor("xhi", [NDP, 128, 2, NK], f8, kind="ExternalInput")
    xlo_d = nc.dram_tensor("xlo", [NDP, 128, 2, NK], f8, kind="ExternalInput")
    wq_d = nc.dram_tensor("wq8", [H, 128, NDP, 2, 2, HD], f8, kind="ExternalInput")
    wk_d = nc.dram_tensor("wk8", [128, NDP, 2, 2, HD], f8, kind="ExternalInput")
    wv_d = nc.dram_tensor("wv8", [128, NDP, 2, 2, HD], f8, kind="ExternalInput")
    wohi_d = nc.dram_tensor("wohi", [4, 128, H, 512], f8, kind="ExternalInput")
    wolo_d = nc.dram_tensor("wolo", [4, 128, NDP, 2, 512], f8, kind="ExternalInput")
    bo_d = nc.dram_tensor("bo", [1, D], fp32, kind="ExternalInput")
    cos_d = nc.dram_tensor("cosT", [ROPE_DIMS, NK], fp32, kind="ExternalInput")
    sin_d = nc.dram_tensor("sinT", [ROPE_DIMS, NK], fp32, kind="ExternalInput")
    msk_d = nc.dram_tensor("masks", [128, 2, 128], bf16, kind="ExternalInput")
    corr_d = nc.dram_tensor("corrA", [128, NQT, 128], bf16, kind="ExternalInput")
    ones_d = nc.dram_tensor("ones", [128, 128], bf16, kind="ExternalInput")
    out_d = nc.dram_tensor("out", [CHUNK, D], fp32, kind="ExternalOutput")

    with tile.TileContext(nc) as tc, ExitStack() as ctx:
        p_const = ctx.enter_context(tc.tile_pool(name="const", bufs=1))
        p_x = ctx.enter_context(tc.tile_pool(name="x", bufs=1))
        p_kv = ctx.enter_context(tc.tile_pool(name="kv", bufs=1))
        p_wq = ctx.enter_context(tc.tile_pool(name="wq", bufs=3))
        p_qt = ctx.enter_context(tc.tile_pool(name="qt", bufs=4))
        p_es = ctx.enter_context(tc.tile_pool(name="es", bufs=6))
        p_red = ctx.enter_context(tc.tile_pool(name="red", bufs=2))
        p_tmp = ctx.enter_context(tc.tile_pool(name="tmp", bufs=4))
        p_otn = ctx.enter_context(tc.tile_pool(name="otn", bufs=1))
        p_wo = ctx.enter_context(tc.tile_pool(name="wo", bufs=4))
        p_ob = ctx.enter_context(tc.tile_pool(name="ob", bufs=3))

        # ---- persistent loads, spread across the three DMA queues ----
        # Activation queue: needed-first order (wk -> x tail -> wv -> cos/sin)
        wk_sb = p_const.tile([128, NDP, 2, 2, HD], f8, tag="wk")
        nc.scalar.dma_start(out=wk_sb, in_=wk_d[:])

        # x hi tiles: one tile per dt-pair, one DMA per dt slice (SP + Act).
        xhi = []
        xlo = []
        for j in range(NDP):
            xhi.append(p_x.tile([128, 2, NK], f8, tag=f"xhi{j}", name=f"xhi{j}"))
            xlo.append(p_x.tile([128, 2, NK], f8, tag=f"xlo{j}", name=f"xlo{j}"))
        xq = {0: nc.sync, 1: nc.sync, 2: nc.scalar, 3: nc.gpsimd,
              4: nc.sync, 5: nc.sync, 6: nc.scalar, 7: nc.gpsimd}
        for j in range(NDP):
            xq[j].dma_start(out=xhi[j][:, 0, :], in_=xhi_d[j, :, 0, :])
            xq[j].dma_start(out=xhi[j][:, 1, :], in_=xhi_d[j, :, 1, :])

        wv_sb = p_const.tile([128, NDP, 2, 2, HD], f8, tag="wv")
        nc.scalar.dma_start(out=wv_sb, in_=wv_d[:])
        cos_sb = p_const.tile([ROPE_DIMS, NK], fp32, tag="cos")
        nc.scalar.dma_start(out=cos_sb, in_=cos_d[:])
        sin_sb = p_const.tile([ROPE_DIMS, NK], fp32, tag="sin")
        nc.scalar.dma_start(out=sin_sb, in_=sin_d[:])

        for j in range(NDP):
            nc.gpsimd.dma_start(out=xlo[j], in_=xlo_d[j])

        # Pool queue: small constants.
        msk_sb = p_const.tile([128, 2, 128], bf16, tag="msk")
        nc.gpsimd.dma_start(out=msk_sb, in_=msk_d[:])
        corr_sb = p_const.tile([128, NQT, 128], bf16, tag="corr")
        nc.gpsimd.dma_start(out=corr_sb, in_=corr_d[:])
        ones_sb = p_const.tile([128, 128], bf16, tag="ones")
        nc.gpsimd.dma_start(out=ones_sb, in_=ones_d[:])
        bias_sb = p_const.tile([128, D], fp32, tag="bias")

        # ---- PE warm-up: tiny dummy matmuls from t~0 to beat the clock ramp
        zt = p_const.tile([128, 16], bf16, tag="zt")
        nc.vector.memset(zt, 0.0)

        # ---- K^T (RoPE'd) and V projections ----
        kt = p_kv.tile([128, NK], bf16, tag="kt")
        v_sb = []
        for s in range(NK // 128):
            v_sb.append(p_kv.tile([128, HD], bf16, tag=f"v{s}", name=f"v{s}"))

        with (
            tc.tile_pool(name="ps_kv", bufs=2, space=bass.MemorySpace.PSUM) as ps_kv,
            tc.tile_pool(name="ps_vv", bufs=5, space=bass.MemorySpace.PSUM) as ps_vv,
            tc.tile_pool(name="ps_dd", bufs=1, space=bass.MemorySpace.PSUM) as ps_dd,
        ):
            psd = ps_dd.tile([16, 16], fp32, tag="ps_dummy")
            for i in range(N_WARMUP):
                nc.tensor.matmul(psd, zt, zt, start=True, stop=True)

            # K: both blocks' hi-matmuls first (xhi-paced), then the xlo
            # corrections — pushes the xlo DMA demand later into the load.
            kps = []
            for nh in range(NK // 512):
                ps = ps_kv.tile([128, 512], fp32, tag="ps_kv")
                cols = slice(nh * 512, (nh + 1) * 512)
                for j in range(NDP):
                    xh = xhi[j][:, :, cols]
                    nc.tensor.matmul(ps, wk_sb[:, j, 0, :, :], xh,
                                     start=(j == 0), stop=False, perf_mode=DR)
                    nc.tensor.matmul(ps, wk_sb[:, j, 1, :, :], xh,
                                     start=False, stop=False, perf_mode=DR)
                kps.append(ps)
            for nh in range(NK // 512):
                cols = slice(nh * 512, (nh + 1) * 512)
                for j in range(NDP):
                    nc.tensor.matmul(kps[nh], wk_sb[:, j, 0, :, :],
                                     xlo[j][:, :, cols],
                                     start=False, stop=(j == NDP - 1),
                                     perf_mode=DR)
                _rope(nc, mybir, p_tmp, kt[:, cols], kps[nh],
                      cos_sb[:, cols], sin_sb[:, cols], fp32, bf16)

            for s in range(NK // 128):
                psv = ps_vv.tile([128, HD], fp32, tag="ps_v")
                cols = slice(s * 128, (s + 1) * 128)
                # V = x^T W: stationary = x slices, moving = Wv slices.
                for j in range(NDP):
                    xh = xhi[j][:, :, cols]
                    nc.tensor.matmul(psv, xh, wv_sb[:, j, 0, :, :],
                                     start=(j == 0), stop=False, perf_mode=DR)
                    nc.tensor.matmul(psv, xh, wv_sb[:, j, 1, :, :],
                                     start=False, stop=False, perf_mode=DR)
                for j in range(NDP):
                    nc.tensor.matmul(psv, xlo[j][:, :, cols], wv_sb[:, j, 0, :, :],
                                     start=False, stop=(j == NDP - 1), perf_mode=DR)
                nc.scalar.activation(v_sb[s], psv, AF.Copy, scale=1.0 / WSCALE)

        # ---- per-head attention ----
        otn8 = p_otn.tile([128, H, 2, CHUNK], f8, tag="otn8")
        qcols = slice(CHUNK, NK)

        with (
            tc.tile_pool(name="ps_q", bufs=2, space=bass.MemorySpace.PSUM) as ps_qp,
            tc.tile_pool(name="ps_s", bufs=2, space=bass.MemorySpace.PSUM) as ps_sp,
            tc.tile_pool(name="ps_o", bufs=2, space=bass.MemorySpace.PSUM) as ps_op,
        ):
            def _normalize(h, otp, rview):
                # deferred one head: off the inter-head critical path
                rflat = rview.rearrange("p t q -> p (t q)")
                tmp32 = p_tmp.tile([128, CHUNK], fp32, tag="otn_tmp", name="otn_tmp")
                nc.vector.tensor_mul(tmp32, otp, rflat)
                nc.scalar.copy(otn8[:, h, 0, :], tmp32)
                nc.gpsimd.tensor_sub(otn8[:, h, 1, :], tmp32, otn8[:, h, 0, :])

            def _qproj(h):
                wq_h = p_wq.tile([128, NDP, 2, 2, HD], f8, tag="wq", name="wq_h")
                nc.sync.dma_start(out=wq_h, in_=wq_d[h])
                psq = ps_qp.tile([128, CHUNK], fp32, tag="ps_q", name="psq")
                _comp_proj(nc, psq, wq_h, xhi, xlo, qcols, True, True, DR)
                return psq

            pending = None
            psq = _qproj(0)
            for h in range(H):
                qt = p_qt.tile([128, CHUNK], bf16, tag="qt")
                _rope(nc, mybir, p_tmp, qt, psq,
                      cos_sb[:, qcols], sin_sb[:, qcols], fp32, bf16)
                if h + 1 < H:
                    psq = _qproj(h + 1)

                otp = ps_op.tile([128, CHUNK], fp32, tag="ps_o")
                rview = p_red.tile([128, NQT, 128], fp32, tag="recip")
                for t in range(NQT):
                    pss = ps_sp.tile([128, NSIG + 1, 128], fp32, tag="ps_s")
                    qsl = qt[:, t * 128:(t + 1) * 128]
                    for sig in range(NSIG):
                        s = t + sig
                        nc.tensor.matmul(
                            pss[:, sig, :], kt[:, s * 128:(s + 1) * 128], qsl,
                            start=True, stop=True,
                        )
                    es = p_es.tile([128, NSIG, 128], bf16, tag="es")
                    nc.scalar.activation(es, pss[:, 0:NSIG, :], AF.Exp,
                                         scale=EXP_SCALE)
                    # boundary masks on sig 0 and sig 4 only (one strided op)
                    esm = es[:]
                    es04 = bass.AP(tensor=esm.tensor, offset=esm.offset,
                                   ap=[list(esm.ap[0]), [4 * 128, 2], [1, 128]])
                    nc.gpsimd.tensor_mul(es04, es04, msk_sb)
                    # denominators: all-ones lhsT (corrA for sig 0) accumulated
                    # into the spare psum slice, replicated across partitions.
                    # middle sigs (1..3) don't depend on the mask op, so they
                    # issue first, hiding the mask latency.
                    for i, sig in enumerate((1, 2, 3, 0, 4)):
                        nc.tensor.matmul(pss[:, NSIG, :],
                                         corr_sb[:, t, :] if sig == 0 else ones_sb,
                                         es[:, sig, :],
                                         start=(i == 0), stop=(i == NSIG - 1))
                    nc.vector.reciprocal(rview[:, t, :], pss[:, NSIG, :])
                    for i, sig in enumerate((1, 2, 3, 0, 4)):
                        nc.tensor.matmul(
                            otp[:, t * 128:(t + 1) * 128],
                            v_sb[t + sig], es[:, sig, :],
                            start=(i == 0), stop=(i == NSIG - 1),
                        )
                if pending is not None:
                    _normalize(*pending)
                pending = (h, otp, rview)
            _normalize(*pending)

            # ---- o-projection + bias (same pool scope: overlaps last heads)
            # chunk big weight DMAs so they never hold the SP queue > ~1us
            for nb in range(4):
                nc.sync.dma_start(
                    out=bias_sb[:, nb * 512:(nb + 1) * 512],
                    in_=bass.AP(tensor=bo_d, offset=nb * 512,
                                ap=[[0, 128], [1, 512]]),
                )
            for n in range(D // 512):
                wo_hi = p_wo.tile([128, H, 512], f8, tag="wo_hi")
                wo_lo = p_wo.tile([128, NDP, 2, 512], f8, tag="wo_lo")
                for q4 in range(4):
                    nc.sync.dma_start(out=wo_hi[:, 4 * q4:4 * q4 + 4, :],
                                      in_=wohi_d[n, :, 4 * q4:4 * q4 + 4, :])
                    nc.sync.dma_start(out=wo_lo[:, 2 * q4:2 * q4 + 2, :, :],
                                      in_=wolo_d[n, :, 2 * q4:2 * q4 + 2, :, :])
                ncols = slice(n * 512, (n + 1) * 512)
                for t in range(NQT):
                    pso = ps_op.tile([128, CHUNK], fp32, tag="ps_o", name="pso")
                    tc_ = slice(t * 128, (t + 1) * 128)
                    for j in range(NDP):
                        whp = wo_hi[:, 2 * j:2 * j + 2, :]
                        nc.tensor.matmul(pso, otn8[:, 2 * j:2 * j + 2, 0, tc_],
                                         whp, start=(j == 0), stop=False,
                                         perf_mode=DR)
                        nc.tensor.matmul(pso, otn8[:, 2 * j:2 * j + 2, 1, tc_],
                                         whp, start=False, stop=False,
                                         perf_mode=DR)
                    for j in range(NDP):
                        nc.tensor.matmul(pso, otn8[:, 2 * j:2 * j + 2, 0, tc_],
                                         wo_lo[:, j, :, :], start=False,
                                         stop=(j == NDP - 1), perf_mode=DR)
                    ob = p_ob.tile([128, 512], fp32, tag="ob")
                    rows = slice(t * 128, (t + 1) * 128)
                    if (n, t) != (3, 3):
                        nc.vector.scalar_tensor_tensor(
                            ob, pso, 1.0 / WSCALE, bias_sb[:, ncols],
                            op0=ALU.mult, op1=ALU.add,
                        )
                        if (n + t) % 2 == 0:
                            nc.sync.dma_start(out=out_d[rows, ncols], in_=ob)
                        else:
                            nc.gpsimd.dma_start(out=out_d[rows, ncols], in_=ob)
                    else:
                        # final tile: split the drain so the last DMA starts
                        # as early as possible
                        for half, q in ((0, nc.sync), (1, nc.gpsimd)):
                            hc = slice(half * 256, (half + 1) * 256)
                            oc = slice(n * 512 + half * 256,
                                       n * 512 + (half + 1) * 256)
                            nc.vector.scalar_tensor_tensor(
                                ob[:, hc], pso[:, hc], 1.0 / WSCALE,
                                bias_sb[:, oc], op0=ALU.mult, op1=ALU.add,
                            )
                            q.dma_start(out=out_d[rows, oc], in_=ob[:, hc])

    nc.compile()
    return nc


def _get_program():
    global _PROGRAM
    if _PROGRAM is None:
        _PROGRAM = _build_program()
    return _PROGRAM


def _q8(a):
    return np.clip(a, -240.0, 240.0).astype(F8)


def _split8(a):
    hi = _q8(a)
    lo = _q8(a - hi.astype(np.float32))
    return hi, lo


def _make_in_maps(x, Wq, Wk, Wv, Wo, bo):
    x = np.asarray(x, np.float32)
    bo_f = np.ascontiguousarray(np.asarray(bo, np.float32).reshape(1, D))

    # --- weights (shared across cores) ---
    qhi, qlo = _split8(WSCALE * np.asarray(Wq, np.float32))
    khi, klo = _split8(WSCALE * np.asarray(Wk, np.float32))
    vhi, vlo = _split8(WSCALE * np.asarray(Wv, np.float32))
    ohi, olo = _split8(WSCALE * np.asarray(Wo, np.float32))

    def warr(hi, lo, M):
        # [D, M] pair -> [128, NDP, 2(hi/lo), 2(dt pair), M]
        w = np.empty((128, NDP, 2, 2, M), F8)
        hi4 = hi.reshape(NDT, 128, M)
        lo4 = lo.reshape(NDT, 128, M)
        for j in range(NDP):
            w[:, j, 0, 0] = hi4[2 * j]
            w[:, j, 0, 1] = hi4[2 * j + 1]
            w[:, j, 1, 0] = lo4[2 * j]
            w[:, j, 1, 1] = lo4[2 * j + 1]
        return np.ascontiguousarray(w)

    wq8 = np.stack([warr(qhi[:, h * HD:(h + 1) * HD], qlo[:, h * HD:(h + 1) * HD], HD)
                    for h in range(H)])
    wk8 = warr(khi, klo, HD)
    wv8 = warr(vhi, vlo, HD)

    wohi = np.empty((4, 128, H, 512), F8)
    wolo = np.empty((4, 128, NDP, 2, 512), F8)
    ohi4 = ohi.reshape(H, 128, D)
    olo4 = olo.reshape(H, 128, D)
    for n in range(4):
        cs = slice(n * 512, (n + 1) * 512)
        for h in range(H):
            wohi[n, :, h] = ohi4[h][:, cs]
        for j in range(NDP):
            wolo[n, :, j, 0] = olo4[2 * j][:, cs]
            wolo[n, :, j, 1] = olo4[2 * j + 1][:, cs]
    wohi = np.ascontiguousarray(wohi)
    wolo = np.ascontiguousarray(wolo)

    inv_freq = np.exp(
        -np.log(np.float32(ROPE_BASE))
        * (np.arange(0, ROPE_DIMS, 2, dtype=np.float32) / np.float32(ROPE_DIMS))
    ).astype(np.float32)

    ones = np.ones((128, 128), BF16)
    # masks: es tile is [key r (partitions), q]; m0 strict upper (r>q),
    # m4 causal lower (r<=q)
    r = np.arange(128)[:, None]
    qi = np.arange(128)[None, :]
    m0_tri = (r > qi).astype(BF16)
    m4 = (r <= qi).astype(BF16)

    in_maps = []
    for c in range(8):
        b, g = divmod(c, 4)
        k_start = 512 * g - 512
        xs = np.zeros((NK, D), np.float32)
        lo_ = max(0, k_start)
        xs[lo_ - k_start:] = x[b, lo_:k_start + NK]
        xT = np.ascontiguousarray(xs.T)                    # [D, NK]
        xh, xl = _split8(xT)
        xhi_a = np.ascontiguousarray(xh.reshape(NDP, 2, 128, NK).transpose(0, 2, 1, 3))
        xlo_a = np.ascontiguousarray(xl.reshape(NDP, 2, 128, NK).transpose(0, 2, 1, 3))

        pos = (k_start + np.arange(NK)).astype(np.float32)
        theta = pos[None, :] * inv_freq[:, None]           # [32, NK]
        cos2 = np.ascontiguousarray(
            np.concatenate([np.cos(theta)] * 2, axis=0).astype(np.float32))
        sin2 = np.ascontiguousarray(
            np.concatenate([-np.sin(theta), np.sin(theta)], axis=0).astype(np.float32))

        msk = np.empty((128, 2, 128), BF16)
        msk[:, 0] = np.ones((128, 128), BF16) if g == 0 else m0_tri
        msk[:, 1] = m4
        corr = np.empty((128, NQT, 128), BF16)
        for t in range(NQT):
            corr[:, t] = np.float32(t - 3) if g == 0 else 1.0

        in_maps.append({
            "xhi": xhi_a, "xlo": xlo_a, "wq8": wq8, "wk8": wk8, "wv8": wv8,
            "wohi": wohi, "wolo": wolo, "bo": bo_f, "cosT": cos2, "sinT": sin2,
            "masks": np.ascontiguousarray(msk), "corrA": np.ascontiguousarray(corr),
            "ones": ones,
        })
    return in_maps


def _unshard(results):
    out = np.zeros((B, L, D), np.float32)
    for c in range(8):
        b, g = divmod(c, 4)
        out[b, CHUNK * g:CHUNK * (g + 1)] = results[c]["out"]
    return out


def kernel(x, Wq, Wk, Wv, Wo, bo):
    from concourse.bass_utils import run_bass_kernel_spmd

    nc = _get_program()
    in_maps = _make_in_maps(x, Wq, Wk, Wv, Wo, bo)
    res = run_bass_kernel_spmd(nc, in_maps, core_ids=list(range(8)))
    return _unshard(res.results)



# revision 60
# speedup vs baseline: 1.0370x; 1.0241x over previous
"""Local sliding-window attention block (MQA + partial RoPE) on 8 TRN2 cores.

Sharding: 2 batches x 4 sequence chunks of 512 queries each (no collectives).
Each core computes q/k/v projections for its chunk (keys include a 512-token
halo), windowed attention (window=512, causal), and the o-projection for its
own query rows; host unshard is pure concatenation.

Projections (Q/K/V/O) run as error-compensated fp8-e4m3 DoubleRow matmuls:
operands are split hi/lo (x = x_hi + x_lo, W*64 = W_hi + W_lo) and the three
product terms hi*hi, hi*lo, lo*hi are paired two-per-instruction across
adjacent contraction slices, giving 0.75x the bf16 cycle count at ~bf16
accuracy. All splits are precomputed host-side except the attention output's
(one Act copy + one DVE sub per head). Scores/PV stay bf16.

Softmax denominators come from all-ones-lhsT matmuls accumulating into a
spare slice of the score PSUM tile (replicated across partitions), replacing
the DVE reduce + GPSIMD all-reduce. Sliding-window masks apply to only the
two boundary key tiles; first-chunk (no-halo) cores are handled with pure
data: all-ones m0, a constant correction lhsT for the sig-0 denominator
matmul, and zero padding in x.

DMAs are spread across the SP, Activation and Pool queues with >=512B
descriptors; dummy matmuls warm the PE clock during the initial load.

CoreSim cost model: ~152 us per core (PE ~96% busy) vs 225 us for the
bf16 baseline; measured rel err vs the fp32 reference: 7.0e-3.
"""

import numpy as np
import ml_dtypes

BF16 = ml_dtypes.bfloat16
F8 = ml_dtypes.float8_e4m3

B, L, D = 2, 2048, 2048
H, HD = 16, 128
ROPE_DIMS, HALF = 64, 32
WINDOW = 512
ROPE_BASE = 10000.0
WSCALE = 64.0                    # weight pre-scale for fp8 range
EXP_SCALE = HD ** -0.5 / (WSCALE * WSCALE)

CHUNK = 512            # queries per core
NK = 1024              # keys (incl. halo) per core
NQT = CHUNK // 128     # 4 local query tiles
NSIG = 5               # key tiles in window per query tile
NDT = D // 128         # 16 contraction tiles over embedding dim
NDP = NDT // 2         # 8 contraction tile-pairs
N_WARMUP = 230  # dummy matmuls to hold PE busy through the p-state ramp

_PROGRAM = None


def _rope(nc, mybir, pool_tmp, out_bf, ps, cos2, sin2m, fp32, bf16):
    """out[0:64] = rotary(ps[0:64]); out[64:128] = ps[64:128]. ps fp32, out bf16.

    cos2 is [64, n] with rows [0:32]==[32:64]==cos(theta); sin2m has rows
    [0:32]==-sin(theta), [32:64]==+sin(theta). Engines can't read across
    partitions, so the half-swap (x2 into rows 0:32, x1 into rows 32:64)
    goes through two partition-shifting DMAs (issued on the Pool queue);
    then out[0:64] = ps[0:64]*cos2 + swapped*sin2m is partition-aligned.
    """
    n = cos2.shape[-1]
    sb64 = pool_tmp.tile([ROPE_DIMS, n], bf16, tag="rope_sb64")
    nc.scalar.copy(sb64, ps[0:ROPE_DIMS])
    ss = pool_tmp.tile([ROPE_DIMS, n], bf16, tag="rope_ss")
    nc.gpsimd.dma_start(out=ss[0:HALF], in_=sb64[HALF:ROPE_DIMS])
    nc.gpsimd.dma_start(out=ss[HALF:ROPE_DIMS], in_=sb64[0:HALF])
    mcos = pool_tmp.tile([ROPE_DIMS, n], fp32, tag="rope_mcos")
    nc.vector.tensor_mul(mcos, ps[0:ROPE_DIMS], cos2)
    mss = pool_tmp.tile([ROPE_DIMS, n], fp32, tag="rope_mss")
    nc.vector.tensor_mul(mss, ss, sin2m)
    nc.vector.tensor_add(out_bf[0:ROPE_DIMS], mcos, mss)
    nc.vector.tensor_copy(out_bf[ROPE_DIMS:HD], ps[ROPE_DIMS:HD])


def _comp_proj(nc, ps, w8, xhi, xlo, cols, n_start, n_stop, DR):
    """Accumulate the compensated-fp8 projection into psum `ps`.

    w8: weight tile [128, NDP, 2(hi/lo), 2(dt pair), M] fp8.
    xhi/xlo: lists of NDP tiles [128, 2(dt pair), NK] fp8.
    For each dt-pair j, three DoubleRow matmuls:
      hi(W) x hi(x),  lo(W) x hi(x),  hi(W) x lo(x).
    """
    for j in range(NDP):
        xh = xhi[j][:, :, cols]
        first = n_start and j == 0
        nc.tensor.matmul(ps, w8[:, j, 0, :, :], xh,
                         start=first, stop=False, perf_mode=DR)
        nc.tensor.matmul(ps, w8[:, j, 1, :, :], xh,
                         start=False, stop=False, perf_mode=DR)
    for j in range(NDP):
        last = n_stop and j == NDP - 1
        nc.tensor.matmul(ps, w8[:, j, 0, :, :], xlo[j][:, :, cols],
                         start=False, stop=last, perf_mode=DR)


def _build_program():
    from contextlib import ExitStack
    import concourse.bass as bass
    import concourse.mybir as mybir
    import concourse.tile as tile
    from concourse import bacc

    fp32 = mybir.dt.float32
    bf16 = mybir.dt.bfloat16
    f8 = mybir.dt.float8e4
    AF = mybir.ActivationFunctionType
    ALU = mybir.AluOpType
    DR = mybir.MatmulPerfMode.DoubleRow

    nc = bacc.Bacc(None, target_bir_lowering=False)

    xhi_d = nc.dram_tensor("xhi", [NDP, 128, 2, NK], f8, kind="ExternalInput")
    xlo_d = nc.dram_tensor("xlo", [NDP, 128, 2, NK], f8, kind="ExternalInput")
    wq_d = nc.dram_tensor("wq8", [H, 128, NDP, 2, 2, HD], f8, kind="ExternalInput")
    wk_d = nc.dram_tensor("wk8", [128, NDP, 2, 2, HD], f8, kind="ExternalInput")
    wv_d = nc.dram_tensor("wv8", [128, NDP, 2, 2, HD], f8, kind="ExternalInput")
    wohi_d = nc.dram_tensor("wohi", [4, 128, H, 512], f8, kind="ExternalInput")
    wolo_d = nc.dram_tensor("wolo", [4, 128, NDP, 2, 512], f8, kind="ExternalInput")
    bo_d = nc.dram_tensor("bo", [1, D], fp32, kind="ExternalInput")
    cos_d = nc.dram_tensor("cosT", [ROPE_DIMS, NK], fp32, kind="ExternalInput")
    sin_d = nc.dram_tensor("sinT", [ROPE_DIMS, NK], fp32, kind="ExternalInput")
    msk_d = nc.dram_tensor("masks", [128, 2, 128], bf16, kind="ExternalInput")
    corr_d = nc.dram_tensor("corrA", [128, NQT, 128], bf16, kind="ExternalInput")
    ones_d = nc.dram_tensor("ones", [128, 128], bf16, kind="ExternalInput")
    out_d = nc.dram_tensor("out", [CHUNK, D], fp32, kind="ExternalOutput")

    with tile.TileContext(nc) as tc, ExitStack() as ctx:
        p_const = ctx.enter_context(tc.tile_pool(name="const", bufs=1))
        p_x = ctx.enter_context(tc.tile_pool(name="x", bufs=1))
        p_kv = ctx.enter_context(tc.tile_pool(name="kv", bufs=1))
        p_wq = ctx.enter_context(tc.tile_pool(name="wq", bufs=3))
        p_qt = ctx.enter_context(tc.tile_pool(name="qt", bufs=4))
        p_es = ctx.enter_context(tc.tile_pool(name="es", bufs=6))
        p_red = ctx.enter_context(tc.tile_pool(name="red", bufs=2))
        p_tmp = ctx.enter_context(tc.tile_pool(name="tmp", bufs=4))
        p_otn = ctx.enter_context(tc.tile_pool(name="otn", bufs=1))
        p_wo = ctx.enter_context(tc.tile_pool(name="wo", bufs=4))
        p_ob = ctx.enter_context(tc.tile_pool(name="ob", bufs=3))

        # ---- persistent loads, spread across the three DMA queues ----
        # Activation queue: needed-first order (wk -> x tail -> wv -> cos/sin)
        wk_sb = p_const.tile([128, NDP, 2, 2, HD], f8, tag="wk")
        nc.scalar.dma_start(out=wk_sb, in_=wk_d[:])
        wv_sb = p_const.tile([128, NDP, 2, 2, HD], f8, tag="wv")
        wv_sb = p_const.tile([128, NDP, 2, 2, HD], f8, tag="wv")

        # x hi tiles: one tile per dt-pair, one DMA per dt slice (SP + Act).
        xhi = []
        xlo = []
        for j in range(NDP):
            xhi.append(p_x.tile([128, 2, NK], f8, tag=f"xhi{j}", name=f"xhi{j}"))
            xlo.append(p_x.tile([128, 2, NK], f8, tag=f"xlo{j}", name=f"xlo{j}"))
        xq = {0: nc.sync, 1: nc.sync, 2: nc.scalar, 3: nc.gpsimd,
              4: nc.sync, 5: nc.sync, 6: nc.scalar, 7: nc.gpsimd}
        for j in range(NDP):
            xq[j].dma_start(out=xhi[j][:, 0, :], in_=xhi_d[j, :, 0, :])
            xq[j].dma_start(out=xhi[j][:, 1, :], in_=xhi_d[j, :, 1, :])
            if j == 3:
                nc.scalar.dma_start(out=wv_sb, in_=wv_d[:])

        cos_sb = p_const.tile([ROPE_DIMS, NK], fp32, tag="cos")
        nc.scalar.dma_start(out=cos_sb, in_=cos_d[:])
        sin_sb = p_const.tile([ROPE_DIMS, NK], fp32, tag="sin")
        nc.scalar.dma_start(out=sin_sb, in_=sin_d[:])

        for j in range(NDP):
            nc.gpsimd.dma_start(out=xlo[j], in_=xlo_d[j])

        # Pool queue: small constants.
        msk_sb = p_const.tile([128, 2, 128], bf16, tag="msk")
        nc.gpsimd.dma_start(out=msk_sb, in_=msk_d[:])
        corr_sb = p_const.tile([128, NQT, 128], bf16, tag="corr")
        nc.gpsimd.dma_start(out=corr_sb, in_=corr_d[:])
        ones_sb = p_const.tile([128, 128], bf16, tag="ones")
        nc.gpsimd.dma_start(out=ones_sb, in_=ones_d[:])
        bias_sb = p_const.tile([128, D], fp32, tag="bias")

        # ---- PE warm-up: tiny dummy matmuls from t~0 to beat the clock ramp
        zt = p_const.tile([128, 16], bf16, tag="zt")
        nc.vector.memset(zt, 0.0)

        # ---- K^T (RoPE'd) and V projections ----
        kt = p_kv.tile([128, NK], bf16, tag="kt")
        v_sb = []
        for s in range(NK // 128):
            v_sb.append(p_kv.tile([128, HD], bf16, tag=f"v{s}", name=f"v{s}"))

        ps_qp = ctx.enter_context(
            tc.tile_pool(name="ps_q", bufs=2, space=bass.MemorySpace.PSUM))

        def _qproj(h):
            wq_h = p_wq.tile([128, NDP, 2, 2, HD], f8, tag="wq", name="wq_h")
            nc.sync.dma_start(out=wq_h, in_=wq_d[h])
            psq = ps_qp.tile([128, CHUNK], fp32, tag="ps_q", name="psq")
            _comp_proj(nc, psq, wq_h, xhi, xlo, slice(CHUNK, NK), True, True, DR)
            return psq

        with (
            tc.tile_pool(name="ps_kv", bufs=2, space=bass.MemorySpace.PSUM) as ps_kv,
            tc.tile_pool(name="ps_vv", bufs=3, space=bass.MemorySpace.PSUM) as ps_vv,
            tc.tile_pool(name="ps_dd", bufs=1, space=bass.MemorySpace.PSUM) as ps_dd,
        ):
            psd = ps_dd.tile([16, 16], fp32, tag="ps_dummy")
            for i in range(N_WARMUP):
                nc.tensor.matmul(psd, zt, zt, start=True, stop=True)

            # K: both blocks' hi-matmuls first (xhi-paced), then the xlo
            # corrections — pushes the xlo DMA demand later into the load.
            kps = []
            for nh in range(NK // 512):
                ps = ps_kv.tile([128, 512], fp32, tag="ps_kv")
                cols = slice(nh * 512, (nh + 1) * 512)
                for j in range(NDP):
                    xh = xhi[j][:, :, cols]
                    nc.tensor.matmul(ps, wk_sb[:, j, 0, :, :], xh,
                                     start=(j == 0), stop=False, perf_mode=DR)
                    nc.tensor.matmul(ps, wk_sb[:, j, 1, :, :], xh,
                                     start=False, stop=False, perf_mode=DR)
                kps.append(ps)
            for nh in range(NK // 512):
                cols = slice(nh * 512, (nh + 1) * 512)
                for j in range(NDP):
                    nc.tensor.matmul(kps[nh], wk_sb[:, j, 0, :, :],
                                     xlo[j][:, :, cols],
                                     start=False, stop=(j == NDP - 1),
                                     perf_mode=DR)
                _rope(nc, mybir, p_tmp, kt[:, cols], kps[nh],
                      cos_sb[:, cols], sin_sb[:, cols], fp32, bf16)

            psq0 = _qproj(0)

            for s in range(NK // 128):
                psv = ps_vv.tile([128, HD], fp32, tag="ps_v")
                cols = slice(s * 128, (s + 1) * 128)
                # V = x^T W: stationary = x slices, moving = Wv slices.
                for j in range(NDP):
                    xh = xhi[j][:, :, cols]
                    nc.tensor.matmul(psv, xh, wv_sb[:, j, 0, :, :],
                                     start=(j == 0), stop=False, perf_mode=DR)
                    nc.tensor.matmul(psv, xh, wv_sb[:, j, 1, :, :],
                                     start=False, stop=False, perf_mode=DR)
                for j in range(NDP):
                    nc.tensor.matmul(psv, xlo[j][:, :, cols], wv_sb[:, j, 0, :, :],
                                     start=False, stop=(j == NDP - 1), perf_mode=DR)
                nc.scalar.activation(v_sb[s], psv, AF.Copy, scale=1.0 / WSCALE)

        # ---- per-head attention ----
        otn8 = p_otn.tile([128, H, 2, CHUNK], f8, tag="otn8")
        qcols = slice(CHUNK, NK)

        with (
            tc.tile_pool(name="ps_s", bufs=2, space=bass.MemorySpace.PSUM) as ps_sp,
            tc.tile_pool(name="ps_o", bufs=2, space=bass.MemorySpace.PSUM) as ps_op,
        ):
            def _normalize(h, otp, rview):
                # deferred one head: off the inter-head critical path
                rflat = rview.rearrange("p t q -> p (t q)")
                tmp32 = p_tmp.tile([128, CHUNK], fp32, tag="otn_tmp", name="otn_tmp")
                nc.vector.tensor_mul(tmp32, otp, rflat)
                nc.scalar.copy(otn8[:, h, 0, :], tmp32)
                nc.gpsimd.tensor_sub(otn8[:, h, 1, :], tmp32, otn8[:, h, 0, :])

            pending = None
            psq = psq0
            for h in range(H):
                qt = p_qt.tile([128, CHUNK], bf16, tag="qt")
                _rope(nc, mybir, p_tmp, qt, psq,
                      cos_sb[:, qcols], sin_sb[:, qcols], fp32, bf16)
                if h + 1 < H:
                    psq = _qproj(h + 1)

                otp = ps_op.tile([128, CHUNK], fp32, tag="ps_o")
                rview = p_red.tile([128, NQT, 128], fp32, tag="recip")
                for t in range(NQT):
                    pss = ps_sp.tile([128, NSIG + 1, 128], fp32, tag="ps_s")
                    qsl = qt[:, t * 128:(t + 1) * 128]
                    for sig in range(NSIG):
                        s = t + sig
                        nc.tensor.matmul(
                            pss[:, sig, :], kt[:, s * 128:(s + 1) * 128], qsl,
                            start=True, stop=True,
                        )
                    es = p_es.tile([128, NSIG, 128], bf16, tag="es")
                    nc.scalar.activation(es, pss[:, 0:NSIG, :], AF.Exp,
                                         scale=EXP_SCALE)
                    # boundary masks on sig 0 and sig 4 only (one strided op)
                    esm = es[:]
                    es04 = bass.AP(tensor=esm.tensor, offset=esm.offset,
                                   ap=[list(esm.ap[0]), [4 * 128, 2], [1, 128]])
                    nc.gpsimd.tensor_mul(es04, es04, msk_sb)
                    # denominators: all-ones lhsT (corrA for sig 0) accumulated
                    # into the spare psum slice, replicated across partitions.
                    # middle sigs (1..3) don't depend on the mask op, so they
                    # issue first, hiding the mask latency.
                    for i, sig in enumerate((1, 2, 3, 0, 4)):
                        nc.tensor.matmul(pss[:, NSIG, :],
                                         corr_sb[:, t, :] if sig == 0 else ones_sb,
                                         es[:, sig, :],
                                         start=(i == 0), stop=(i == NSIG - 1))
                    nc.vector.reciprocal(rview[:, t, :], pss[:, NSIG, :])
                    for i, sig in enumerate((1, 2, 3, 0, 4)):
                        nc.tensor.matmul(
                            otp[:, t * 128:(t + 1) * 128],
                            v_sb[t + sig], es[:, sig, :],
                            start=(i == 0), stop=(i == NSIG - 1),
                        )
                if pending is not None:
                    _normalize(*pending)
                pending = (h, otp, rview)
            _normalize(*pending)

            # ---- o-projection + bias (same pool scope: overlaps last heads)
            # chunk big weight DMAs so they never hold the SP queue > ~1us
            for nb in range(4):
                nc.sync.dma_start(
                    out=bias_sb[:, nb * 512:(nb + 1) * 512],
                    in_=bass.AP(tensor=bo_d, offset=nb * 512,
                                ap=[[0, 128], [1, 512]]),
                )
            for n in range(D // 512):
                wo_hi = p_wo.tile([128, H, 512], f8, tag="wo_hi")
                wo_lo = p_wo.tile([128, NDP, 2, 512], f8, tag="wo_lo")
                for q4 in range(4):
                    nc.sync.dma_start(out=wo_hi[:, 4 * q4:4 * q4 + 4, :],
                                      in_=wohi_d[n, :, 4 * q4:4 * q4 + 4, :])
                    nc.sync.dma_start(out=wo_lo[:, 2 * q4:2 * q4 + 2, :, :],
                                      in_=wolo_d[n, :, 2 * q4:2 * q4 + 2, :, :])
                ncols = slice(n * 512, (n + 1) * 512)
                for t in range(NQT):
                    tc_ = slice(t * 128, (t + 1) * 128)
                    rows = slice(t * 128, (t + 1) * 128)
                    ob = p_ob.tile([128, 512], fp32, tag="ob")
                    # the final tile runs as two independent half-column
                    # accumulation groups so the first half's drain + DMA
                    # hide behind the second half's matmuls
                    halves = 4 if (n, t) == (3, 3) else 1
                    for hf in range(halves):
                        w = 512 // halves
                        wc = slice(hf * w, (hf + 1) * w)
                        oc = slice(n * 512 + hf * w, n * 512 + (hf + 1) * w)
                        if (n * NQT + t + hf) % 2 == 0:
                            pso = ps_qp.tile([128, CHUNK], fp32, tag="ps_q",
                                             name="pso")
                        else:
                            pso = ps_op.tile([128, CHUNK], fp32, tag="ps_o",
                                             name="pso")
                        for j in range(NDP):
                            whp = wo_hi[:, 2 * j:2 * j + 2, wc]
                            nc.tensor.matmul(
                                pso[:, 0:w], otn8[:, 2 * j:2 * j + 2, 0, tc_],
                                whp, start=(j == 0), stop=False, perf_mode=DR)
                            nc.tensor.matmul(
                                pso[:, 0:w], otn8[:, 2 * j:2 * j + 2, 1, tc_],
                                whp, start=False, stop=False, perf_mode=DR)
                        for j in range(NDP):
                            nc.tensor.matmul(
                                pso[:, 0:w], otn8[:, 2 * j:2 * j + 2, 0, tc_],
                                wo_lo[:, j, :, wc], start=False,
                                stop=(j == NDP - 1), perf_mode=DR)
                        nc.vector.scalar_tensor_tensor(
                            ob[:, wc], pso[:, 0:w], 1.0 / WSCALE,
                            bias_sb[:, oc], op0=ALU.mult, op1=ALU.add,
                        )
                        if (n + t + hf) % 2 == 0:
                            nc.sync.dma_start(out=out_d[rows, oc],
                                              in_=ob[:, wc])
                        else:
                            nc.gpsimd.dma_start(out=out_d[rows, oc],
                                                in_=ob[:, wc])

    nc.compile()
    return nc


def _get_program():
    global _PROGRAM
    if _PROGRAM is None:
        _PROGRAM = _build_program()
    return _PROGRAM


def _q8(a):
    return np.clip(a, -240.0, 240.0).astype(F8)


def _split8(a):
    hi = _q8(a)
    lo = _q8(a - hi.astype(np.float32))
    return hi, lo


def _make_in_maps(x, Wq, Wk, Wv, Wo, bo):
    x = np.asarray(x, np.float32)
    bo_f = np.ascontiguousarray(np.asarray(bo, np.float32).reshape(1, D))

    # --- weights (shared across cores) ---
    qhi, qlo = _split8(WSCALE * np.asarray(Wq, np.float32))
    khi, klo = _split8(WSCALE * np.asarray(Wk, np.float32))
    vhi, vlo = _split8(WSCALE * np.asarray(Wv, np.float32))
    ohi, olo = _split8(WSCALE * np.asarray(Wo, np.float32))

    def warr(hi, lo, M):
        # [D, M] pair -> [128, NDP, 2(hi/lo), 2(dt pair), M]
        w = np.empty((128, NDP, 2, 2, M), F8)
        hi4 = hi.reshape(NDT, 128, M)
        lo4 = lo.reshape(NDT, 128, M)
        for j in range(NDP):
            w[:, j, 0, 0] = hi4[2 * j]
            w[:, j, 0, 1] = hi4[2 * j + 1]
            w[:, j, 1, 0] = lo4[2 * j]
            w[:, j, 1, 1] = lo4[2 * j + 1]
        return np.ascontiguousarray(w)

    wq8 = np.stack([warr(qhi[:, h * HD:(h + 1) * HD], qlo[:, h * HD:(h + 1) * HD], HD)
                    for h in range(H)])
    wk8 = warr(khi, klo, HD)
    wv8 = warr(vhi, vlo, HD)

    wohi = np.empty((4, 128, H, 512), F8)
    wolo = np.empty((4, 128, NDP, 2, 512), F8)
    ohi4 = ohi.reshape(H, 128, D)
    olo4 = olo.reshape(H, 128, D)
    for n in range(4):
        cs = slice(n * 512, (n + 1) * 512)
        for h in range(H):
            wohi[n, :, h] = ohi4[h][:, cs]
        for j in range(NDP):
            wolo[n, :, j, 0] = olo4[2 * j][:, cs]
            wolo[n, :, j, 1] = olo4[2 * j + 1][:, cs]
    wohi = np.ascontiguousarray(wohi)
    wolo = np.ascontiguousarray(wolo)

    inv_freq = np.exp(
        -np.log(np.float32(ROPE_BASE))
        * (np.arange(0, ROPE_DIMS, 2, dtype=np.float32) / np.float32(ROPE_DIMS))
    ).astype(np.float32)

    ones = np.ones((128, 128), BF16)
    # masks: es tile is [key r (partitions), q]; m0 strict upper (r>q),
    # m4 causal lower (r<=q)
    r = np.arange(128)[:, None]
    qi = np.arange(128)[None, :]
    m0_tri = (r > qi).astype(BF16)
    m4 = (r <= qi).astype(BF16)

    in_maps = []
    for c in range(8):
        b, g = divmod(c, 4)
        k_start = 512 * g - 512
        xs = np.zeros((NK, D), np.float32)
        lo_ = max(0, k_start)
        xs[lo_ - k_start:] = x[b, lo_:k_start + NK]
        xT = np.ascontiguousarray(xs.T)                    # [D, NK]
        xh, xl = _split8(xT)
        xhi_a = np.ascontiguousarray(xh.reshape(NDP, 2, 128, NK).transpose(0, 2, 1, 3))
        xlo_a = np.ascontiguousarray(xl.reshape(NDP, 2, 128, NK).transpose(0, 2, 1, 3))

        pos = (k_start + np.arange(NK)).astype(np.float32)
        theta = pos[None, :] * inv_freq[:, None]           # [32, NK]
        cos2 = np.ascontiguousarray(
            np.concatenate([np.cos(theta)] * 2, axis=0).astype(np.float32))
        sin2 = np.ascontiguousarray(
            np.concatenate([-np.sin(theta), np.sin(theta)], axis=0).astype(np.float32))

        msk = np.empty((128, 2, 128), BF16)
        msk[:, 0] = np.ones((128, 128), BF16) if g == 0 else m0_tri
        msk[:, 1] = m4
        corr = np.empty((128, NQT, 128), BF16)
        for t in range(NQT):
            corr[:, t] = np.float32(t - 3) if g == 0 else 1.0

        in_maps.append({
            "xhi": xhi_a, "xlo": xlo_a, "wq8": wq8, "wk8": wk8, "wv8": wv8,
            "wohi": wohi, "wolo": wolo, "bo": bo_f, "cosT": cos2, "sinT": sin2,
            "masks": np.ascontiguousarray(msk), "corrA": np.ascontiguousarray(corr),
            "ones": ones,
        })
    return in_maps


def _unshard(results):
    out = np.zeros((B, L, D), np.float32)
    for c in range(8):
        b, g = divmod(c, 4)
        out[b, CHUNK * g:CHUNK * (g + 1)] = results[c]["out"]
    return out


def kernel(x, Wq, Wk, Wv, Wo, bo):
    from concourse.bass_utils import run_bass_kernel_spmd

    nc = _get_program()
    in_maps = _make_in_maps(x, Wq, Wk, Wv, Wo, bo)
    res = run_bass_kernel_spmd(nc, in_maps, core_ids=list(range(8)))
    return _unshard(res.results)

